# revision 8
# baseline (speedup 1.0000x reference)
"""Trainium2 Bass kernel for GraphStateRepresentation (GNN message passing).

Algorithmic notes:
  * The reference's inner NSTEPS loop is a fixed point: feat_flat never
    changes inside it, so msg/a/feat_out are identical every iteration ->
    one message-passing step suffices.
  * The per-edge [D,D] weight gather is recast as dense matmuls:
        a = sum_k A_k @ (h @ W_k.T)
    with A_k[dst,src] per-(b,t,edgetype) count matrices (edge mask folded
    in) built on host from the integer edge list.
  * Sharding: 8 cores = batch(4) x time-half(2). SPMD runs one program:
    6 outer-GRU recurrence steps with message passing (the expensive
    part) on program steps 3..5 only. th=1 cores feed t=0..5 naturally;
    th=0 cores zero program steps 0..2 (an explicit x-mask forces the
    projected input to exactly 0, and a bias-free GRU maps (x=0,h=0)->0)
    and place their real t=0..2 at program steps 3..5. No collectives.
  * Layout: feature-major [D=100 partitions, nodes free] so every linear
    layer is one matmul per <=128-wide output block; GRU gate pair sums
    (x@Wih + h@Whh) accumulate in PSUM for free.
"""

import sys
import types

import numpy as np
import ml_dtypes

BS, T, N, E = 4, 6, 400, 3000
D = 100
NS = 30
NOBJ = 300
NET = 10
NCORES = 8
TH = T // 2  # active timesteps per core
BF16 = ml_dtypes.bfloat16


def _install_ntff_shim():
    """Provide antenv.axon_hooks (absent on this image) so trace=True /
    BASS_TRACE=1 profiling works instead of crashing. Silent no-op on
    any failure."""
    try:
        if "antenv.axon_hooks" in sys.modules:
            return
        m = types.ModuleType("antenv.axon_hooks")
        m._hook = None
        m.set_axon_ntff_profile_hook = lambda h: setattr(m, "_hook", h)
        m.get_axon_ntff_profile_hook = lambda: m._hook
        sys.modules["antenv.axon_hooks"] = m
        import antenv

        antenv.axon_hooks = m
        from trn_agent_boot.trn_boot import _ntff_profile_via_ctypes

        hook = _ntff_profile_via_ctypes("/opt/axon/libaxon_pjrt.so")
        if hook is not None:
            m.set_axon_ntff_profile_hook(hook)
    except Exception:
        pass


_NC_CACHE = {}
LAST_RESULT = None


def _build_nc():
    import concourse.tile as tile
    from concourse import bacc, mybir

    f32 = mybir.dt.float32
    bf16 = mybir.dt.bfloat16
    AF = mybir.ActivationFunctionType
    ALU = mybir.AluOpType

    nc = bacc.Bacc("TRN2", target_bir_lowering=False, debug=False,
                   num_devices=NCORES)

    # ---- DRAM I/O (identical shapes on every core; data differs) ----
    oh_d = nc.dram_tensor("oh", [D, T * 3 * N], bf16, kind="ExternalInput")
    st_d = nc.dram_tensor("st", [NS, T * N], bf16, kind="ExternalInput")
    xm_d = nc.dram_tensor("xm", [D, T], f32, kind="ExternalInput")
    visb_d = nc.dram_tensor("visb", [D, TH * N], f32, kind="ExternalInput")
    a_d = nc.dram_tensor("a4", [4, D, TH * NET * N], bf16,
                         kind="ExternalInput")
    ep_d = nc.dram_tensor("ep", [D, 3 * D], bf16, kind="ExternalInput")
    sp_d = nc.dram_tensor("sp", [NS, D], bf16, kind="ExternalInput")
    c2t_d = nc.dram_tensor("c2t", [D, D], bf16, kind="ExternalInput")
    b1_d = nc.dram_tensor("b1", [D, 1], f32, kind="ExternalInput")
    c2b_d = nc.dram_tensor("c2b", [D, 1], f32, kind="ExternalInput")
    owih_d = nc.dram_tensor("owih", [D, 3 * D], f32, kind="ExternalInput")
    owhh_d = nc.dram_tensor("owhh", [D, 3 * D], f32, kind="ExternalInput")
    gwih_d = nc.dram_tensor("gwih", [D, 3 * D], f32, kind="ExternalInput")
    gwhh_d = nc.dram_tensor("gwhh", [D, 3 * D], f32, kind="ExternalInput")
    gb_d = nc.dram_tensor("gb", [D, 4], f32, kind="ExternalInput")
    wct_d = nc.dram_tensor("wct", [D, NET * D], bf16, kind="ExternalInput")
    ident_d = nc.dram_tensor("ident", [D, D], f32, kind="ExternalInput")
    visn_d = nc.dram_tensor("visn", [D, TH * 4], f32, kind="ExternalInput")
    outr_d = nc.dram_tensor("outr", [TH, N, D], f32, kind="ExternalOutput")
    outg_d = nc.dram_tensor("outg", [D, TH], f32, kind="ExternalOutput")

    with tile.TileContext(nc) as tc:
        with (
            tc.tile_pool(name="const", bufs=1) as cpool,
            tc.tile_pool(name="wk", bufs=2) as wk,
            tc.tile_pool(name="zsb", bufs=1) as zpool,
            tc.tile_pool(name="pg", bufs=1, space="PSUM") as pg,
            tc.tile_pool(name="pzz", bufs=1, space="PSUM") as pzz,
            tc.tile_pool(name="pa", bufs=2, space="PSUM") as pa,
        ):
            # ---- persistent tiles + param loads ----
            oh_sb = cpool.tile([D, T * 3 * N], bf16)
            st_sb = cpool.tile([NS, T * N], bf16)
            xm_sb = cpool.tile([D, T], f32)
            visb_sb = cpool.tile([D, TH * N], f32)
            a_sb = [cpool.tile([D, TH * NET * N], bf16, tag=f"a{c}",
                               name=f"a_sb{c}")
                    for c in range(4)]
            ep_sb = cpool.tile([D, 3 * D], bf16)
            sp_sb = cpool.tile([NS, D], bf16)
            c2t_sb = cpool.tile([D, D], bf16)
            b1_sb = cpool.tile([D, 1], f32)
            c2b_sb = cpool.tile([D, 1], f32)
            owih_sb = cpool.tile([D, 3 * D], f32)
            owhh_sb = cpool.tile([D, 3 * D], f32)
            gwih_sb = cpool.tile([D, 3 * D], f32)
            gwhh_sb = cpool.tile([D, 3 * D], f32)
            gb_sb = cpool.tile([D, 4], f32)
            wct_sb = cpool.tile([D, NET * D], bf16)
            ident_sb = cpool.tile([D, D], f32)
            visn_sb = cpool.tile([D, TH * 4], f32)
            xf_sb = cpool.tile([D, T * N], f32)
            hf = cpool.tile([D, N], f32)
            gacc = cpool.tile([D, TH], f32)

            for sb, dr in [
                (oh_sb, oh_d), (st_sb, st_d), (xm_sb, xm_d),
                (visb_sb, visb_d), (ep_sb, ep_d), (sp_sb, sp_d),
                (c2t_sb, c2t_d), (b1_sb, b1_d), (c2b_sb, c2b_d),
                (owih_sb, owih_d), (owhh_sb, owhh_d), (gwih_sb, gwih_d),
                (gwhh_sb, gwhh_d), (gb_sb, gb_d), (wct_sb, wct_d),
                (ident_sb, ident_d), (visn_sb, visn_d),
            ]:
                nc.sync.dma_start(sb[:], dr.ap())
            # A: per (chunk, tt) pieces so tt=0 message passing can start
            # before the whole 9.6MB has landed.
            SEG = NET * N
            for c in range(4):
                for tt in range(TH):
                    nc.sync.dma_start(
                        a_sb[c][:, tt * SEG:(tt + 1) * SEG],
                        a_d.ap()[c, :, tt * SEG:(tt + 1) * SEG],
                    )

            nc.vector.memset(hf[:], 0.0)

            # ---- input projection x_t for all 6 program steps ----
            for t in range(T):
                xp = pg.tile([D, N], f32, tag="pr")
                for c in range(3):
                    nc.tensor.matmul(
                        xp[:], ep_sb[:, c * D:(c + 1) * D],
                        oh_sb[:, (t * 3 + c) * N:(t * 3 + c + 1) * N],
                        start=(c == 0), stop=False,
                    )
                nc.tensor.matmul(xp[:], sp_sb[:],
                                 st_sb[:, t * N:(t + 1) * N],
                                 start=False, stop=True)
                x1 = wk.tile([D, N], bf16, tag="x1")
                nc.scalar.activation(x1[:], xp[:], AF.Relu, bias=b1_sb[:])
                xq = pg.tile([D, N], f32, tag="pz")
                nc.tensor.matmul(xq[:], c2t_sb[:], x1[:],
                                 start=True, stop=True)
                x2 = wk.tile([D, N], f32, tag="x2")
                nc.scalar.activation(x2[:], xq[:], AF.Identity,
                                     bias=c2b_sb[:])
                # x-mask (exact zero for th=0 cores' padding steps)
                nc.vector.tensor_scalar_mul(
                    xf_sb[:, t * N:(t + 1) * N], x2[:], xm_sb[:, t:t + 1])

            # ---- recurrence + message passing ----
            for t in range(T):
                # outer GRU: h = gru(x_t, h)  (bias-free)
                pr = pg.tile([D, N], f32, tag="pr")
                pz = pg.tile([D, N], f32, tag="pz")
                pni = pg.tile([D, N], f32, tag="pni")
                pnh = pg.tile([D, N], f32, tag="pnh")
                xt = xf_sb[:, t * N:(t + 1) * N]
                nc.tensor.matmul(pr[:], owih_sb[:, 0:D], xt,
                                 start=True, stop=False)
                nc.tensor.matmul(pr[:], owhh_sb[:, 0:D], hf[:],
                                 start=False, stop=True)
                nc.tensor.matmul(pz[:], owih_sb[:, D:2 * D], xt,
                                 start=True, stop=False)
                nc.tensor.matmul(pz[:], owhh_sb[:, D:2 * D], hf[:],
                                 start=False, stop=True)
                nc.tensor.matmul(pni[:], owih_sb[:, 2 * D:3 * D], xt,
                                 start=True, stop=True)
                nc.tensor.matmul(pnh[:], owhh_sb[:, 2 * D:3 * D], hf[:],
                                 start=True, stop=True)
                r_ = wk.tile([D, N], f32, tag="r")
                z_ = wk.tile([D, N], f32, tag="z")
                nc.scalar.activation(r_[:], pr[:], AF.Sigmoid)
                nc.scalar.activation(z_[:], pz[:], AF.Sigmoid)
                t1 = wk.tile([D, N], f32, tag="t1")
                nc.vector.tensor_mul(t1[:], r_[:], pnh[:])
                s1 = wk.tile([D, N], f32, tag="s1")
                nc.vector.tensor_add(s1[:], t1[:], pni[:])
                nn_ = wk.tile([D, N], f32, tag="nn")
                nc.scalar.activation(nn_[:], s1[:], AF.Tanh)
                d1 = wk.tile([D, N], f32, tag="d1")
                nc.vector.tensor_sub(d1[:], hf[:], nn_[:])
                zd = wk.tile([D, N], f32, tag="zd")
                nc.vector.tensor_mul(zd[:], z_[:], d1[:])
                nc.vector.tensor_add(hf[:], nn_[:], zd[:])

                if t < 3:
                    continue
                tt = t - 3

                # message passing on h
                hb = wk.tile([D, N], bf16, tag="x1")
                nc.scalar.activation(hb[:], hf[:], AF.Copy)
                zt = zpool.tile([N // 4, 4 * NET * D], bf16, tag="zs")
                for c in range(4):
                    for hh in range(2):
                        zp = pzz.tile([N // 4, NET * D // 2], f32)
                        nc.tensor.matmul(
                            zp[:], hb[:, c * D:(c + 1) * D],
                            wct_sb[:, hh * 500:(hh + 1) * 500],
                            start=True, stop=True)
                        dc = c * NET * D + hh * 500
                        if (c + hh) % 2 == 0:
                            nc.vector.tensor_copy(zt[:, dc:dc + 500], zp[:])
                        else:
                            nc.scalar.activation(zt[:, dc:dc + 500], zp[:],
                                                 AF.Copy)
                ap_ = pa.tile([D, N], f32, tag="ap")
                nmm = 4 * NET
                i = 0
                for c in range(4):
                    for k in range(NET):
                        nc.tensor.matmul(
                            ap_[:], zt[:, c * NET * D + k * D:
                                       c * NET * D + (k + 1) * D],
                            a_sb[c][:, (tt * NET + k) * N:
                                    (tt * NET + k + 1) * N],
                            start=(i == 0), stop=(i == nmm - 1),
                        )
                        i += 1
                af = wk.tile([D, N], f32, tag="af")
                nc.scalar.activation(af[:], ap_[:], AF.Copy)

                # inner GRU: feat = gru(a, h)  (with biases)
                qr = pg.tile([D, N], f32, tag="pr")
                qz = pg.tile([D, N], f32, tag="pz")
                qni = pg.tile([D, N], f32, tag="pni")
                qnh = pg.tile([D, N], f32, tag="pnh")
                nc.tensor.matmul(qr[:], gwih_sb[:, 0:D], af[:],
                                 start=True, stop=False)
                nc.tensor.matmul(qr[:], gwhh_sb[:, 0:D], hf[:],
                                 start=False, stop=True)
                nc.tensor.matmul(qz[:], gwih_sb[:, D:2 * D], af[:],
                                 start=True, stop=False)
                nc.tensor.matmul(qz[:], gwhh_sb[:, D:2 * D], hf[:],
                                 start=False, stop=True)
                nc.tensor.matmul(qni[:], gwih_sb[:, 2 * D:3 * D], af[:],
                                 start=True, stop=True)
                nc.tensor.matmul(qnh[:], gwhh_sb[:, 2 * D:3 * D], hf[:],
                                 start=True, stop=True)
                gr = wk.tile([D, N], f32, tag="r")
                gz = wk.tile([D, N], f32, tag="z")
                nc.scalar.activation(gr[:], qr[:], AF.Sigmoid,
                                     bias=gb_sb[:, 0:1])
                nc.scalar.activation(gz[:], qz[:], AF.Sigmoid,
                                     bias=gb_sb[:, 1:2])
                hn = wk.tile([D, N], f32, tag="hn")
                nc.scalar.activation(hn[:], qnh[:], AF.Identity,
                                     bias=gb_sb[:, 3:4])
                t2 = wk.tile([D, N], f32, tag="t1")
                nc.vector.tensor_mul(t2[:], gr[:], hn[:])
                s2 = wk.tile([D, N], f32, tag="s1")
                nc.vector.tensor_add(s2[:], t2[:], qni[:])
                gn = wk.tile([D, N], f32, tag="nn")
                nc.scalar.activation(gn[:], s2[:], AF.Tanh,
                                     bias=gb_sb[:, 2:3])
                d2 = wk.tile([D, N], f32, tag="d1")
                nc.vector.tensor_sub(d2[:], hf[:], gn[:])
                zd2 = wk.tile([D, N], f32, tag="zd")
                nc.vector.tensor_mul(zd2[:], gz[:], d2[:])
                feat = wk.tile([D, N], f32, tag="feat")
                nc.vector.tensor_add(feat[:], gn[:], zd2[:])

                # node reprs (feat * vis) and global partial
                # (sum_n feat*vis^2) in two DVE ops
                vist = visb_sb[:, tt * N:(tt + 1) * N]
                reprs = wk.tile([D, N], f32, tag="reprs")
                nc.vector.tensor_mul(reprs[:], feat[:], vist)

                # transpose to node-major, store, and accumulate the
                # global readout sum_n reprs[n,:]*vis[n] on the PE
                gp = pa.tile([D, 1], f32, tag="gp", bufs=1)
                for c in range(4):
                    tp = pa.tile([D, D], f32, tag="ap")
                    nc.tensor.transpose(tp[:], reprs[:, c * D:(c + 1) * D],
                                        ident_sb[:])
                    ro = wk.tile([D, D], f32, tag="ro")
                    nc.scalar.activation(ro[:], tp[:], AF.Copy)
                    nc.sync.dma_start(
                        outr_d.ap()[tt, c * D:(c + 1) * D, :], ro[:])
                    nc.tensor.matmul(gp[:], ro[:],
                                     visn_sb[:, tt * 4 + c:tt * 4 + c + 1],
                                     start=(c == 0), stop=(c == 3))
                nc.scalar.activation(gacc[:, tt:tt + 1], gp[:], AF.Copy)

            nc.sync.dma_start(outg_d.ap(), gacc[:])

    nc.compile()
    return nc


def _prep_inputs(inputs):
    f32 = np.float32
    cn = np.asarray(inputs["class_names"])
    states = np.asarray(inputs["states"], f32)
    edges = np.asarray(inputs["edges"])
    etyp = np.asarray(inputs["edge_types"])
    vis = np.asarray(inputs["visibility"], f32)
    mask = np.asarray(inputs["mask_edges"], f32)
    obj_emb = np.asarray(inputs["obj_emb"], f32)
    state_W = np.asarray(inputs["state_W"], f32)
    state_b = np.asarray(inputs["state_b"], f32)
    c1_W = np.asarray(inputs["c1_W"], f32)
    c1_b = np.asarray(inputs["c1_b"], f32)
    c2_W = np.asarray(inputs["c2_W"], f32)
    c2_b = np.asarray(inputs["c2_b"], f32)
    edge_embed = np.asarray(inputs["edge_embed"], f32)
    g_wih = np.asarray(inputs["g_wih"], f32)
    g_whh = np.asarray(inputs["g_whh"], f32)
    g_bih = np.asarray(inputs["g_bih"], f32)
    g_bhh = np.asarray(inputs["g_bhh"], f32)
    o_wih = np.asarray(inputs["o_wih"], f32)
    o_whh = np.asarray(inputs["o_whh"], f32)

    # folded params (parameter-only algebra)
    c1a, c1b = c1_W[:, :D], c1_W[:, D:]
    ep = (obj_emb @ c1a.T).reshape(3, D, D).transpose(1, 0, 2).reshape(D, 3 * D)
    sp = (c1b @ state_W).T  # [NS, D]
    b1 = (c1_b + c1b @ state_b)[:, None]
    wct = edge_embed.reshape(NET, D, D).transpose(2, 0, 1).reshape(D, NET * D)
    gb = np.stack([
        g_bih[:D] + g_bhh[:D],
        g_bih[D:2 * D] + g_bhh[D:2 * D],
        g_bih[2 * D:],
        g_bhh[2 * D:],
    ], axis=1)

    params = dict(
        ep=ep.astype(BF16), sp=sp.astype(BF16),
        c2t=np.ascontiguousarray(c2_W.T).astype(BF16),
        b1=b1.astype(f32), c2b=c2_b[:, None].astype(f32),
        owih=np.ascontiguousarray(o_wih.T).astype(f32),
        owhh=np.ascontiguousarray(o_whh.T).astype(f32),
        gwih=np.ascontiguousarray(g_wih.T).astype(f32),
        gwhh=np.ascontiguousarray(g_whh.T).astype(f32),
        gb=gb.astype(f32), wct=wct.astype(BF16),
        ident=np.eye(D, dtype=f32),
    )

    in_maps = []
    t_rep = np.repeat(np.arange(T), E)
    for core in range(NCORES):
        b, th = core // 2, core % 2
        tsel = np.arange(th * TH, th * TH + TH)  # real t's owned

        # one-hot classes, feature(vocab)-major; th=0 pads steps 0..2
        oh = np.zeros((D, T, 3, N), BF16)
        stf = np.zeros((NS, T, N), BF16)
        xm = np.zeros((D, T), f32)
        for ps in range(T):  # program step
            if th == 1 and ps < TH:
                rt = ps  # recurrence prefix: real t=0..2
            elif th == 1:
                rt = ps
            elif th == 0 and ps >= TH:
                rt = ps - TH  # real t = 0..2 at program steps 3..5
            else:
                continue  # th=0 padding steps: stay zero
            onehot = (cn[b, rt][:, None] ==
                      np.arange(NOBJ)[None, :])  # [N, NOBJ]
            oh[:, ps] = onehot.T.reshape(3, D, N).transpose(1, 0, 2)
            stf[:, ps] = states[b, rt].T
            xm[:, ps] = 1.0
        # adjacency counts for owned t's
        a4 = np.zeros((4, D, TH, NET, N), f32)
        cnt = np.zeros((TH, NET, N, N), f32)
        trl = np.repeat(np.arange(TH), E)
        ksel = etyp[b, tsel].reshape(-1)
        srcsel = edges[b, tsel, :, 0].reshape(-1)
        dstsel = edges[b, tsel, :, 1].reshape(-1)
        msel = mask[b, tsel].reshape(-1)
        np.add.at(cnt, (trl, ksel, srcsel, dstsel), msel)
        # [tt,k,src,dst] -> [src, tt, k, dst] -> chunked on src
        a4 = cnt.transpose(2, 0, 1, 3).reshape(4, D, TH * NET * N)

        visb = np.broadcast_to(
            vis[b, tsel][None, :, :], (D, TH, N)).reshape(D, TH * N)

        im = dict(params)
        im["oh"] = oh.reshape(D, T * 3 * N)
        im["st"] = stf.reshape(NS, T * N)
        im["xm"] = xm
        im["visb"] = np.ascontiguousarray(visb, f32)
        visn = vis[b, tsel].reshape(TH * 4, D).T
        im["visn"] = np.ascontiguousarray(visn, f32)
        im["a4"] = np.ascontiguousarray(a4).astype(BF16)
        in_maps.append(im)
    return in_maps, vis


def kernel(**inputs):
    global LAST_RESULT
    _install_ntff_shim()
    from concourse.bass_utils import run_bass_kernel_spmd

    if "nc" not in _NC_CACHE:
        _NC_CACHE["nc"] = _build_nc()
    nc = _NC_CACHE["nc"]

    in_maps, vis = _prep_inputs(inputs)
    res = run_bass_kernel_spmd(nc, in_maps, list(range(NCORES)))
    LAST_RESULT = res

    node_reprs = np.empty((BS, T, N, D), np.float32)
    global_repr = np.empty((BS, T, D), np.float32)
    denom = vis.sum(-1) + 1e-9  # [BS, T]
    for core in range(NCORES):
        b, th = core // 2, core % 2
        out = res.results[core]
        node_reprs[b, th * TH:(th + 1) * TH] = out["outr"]
        gp = out["outg"]  # [D, TH]
        for tt in range(TH):
            t = th * TH + tt
            global_repr[b, t] = gp[:, tt] / denom[b, t]
    return node_reprs, global_repr


# revision 10
# speedup vs baseline: 1.0891x; 1.0891x over previous
"""Trainium2 Bass kernel for GraphStateRepresentation (GNN message passing).

Algorithmic notes:
  * The reference's inner NSTEPS loop is a fixed point: feat_flat never
    changes inside it, so msg/a/feat_out are identical every iteration ->
    one message-passing step suffices.
  * The per-edge [D,D] weight gather is recast as dense matmuls:
        a = sum_k A_k @ (h @ W_k.T)
    with A_k[dst,src] per-(b,t,edgetype) count matrices (edge mask folded
    in) built on host from the integer edge list.
  * Sharding: 8 cores = batch(4) x time-half(2). SPMD runs one program:
    6 outer-GRU recurrence steps with message passing (the expensive
    part) on program steps 3..5 only. th=1 cores feed t=0..5 naturally;
    th=0 cores zero program steps 0..2 (an explicit x-mask forces the
    projected input to exactly 0, and a bias-free GRU maps (x=0,h=0)->0)
    and place their real t=0..2 at program steps 3..5. No collectives.
  * Layout: feature-major [D=100 partitions, nodes free] so every linear
    layer is one matmul per <=128-wide output block; GRU gate pair sums
    (x@Wih + h@Whh) accumulate in PSUM for free.
"""

import sys
import types

import numpy as np
import ml_dtypes

BS, T, N, E = 4, 6, 400, 3000
D = 100
NS = 30
NOBJ = 300
NET = 10
NCORES = 8
TH = T // 2  # active timesteps per core
BF16 = ml_dtypes.bfloat16


def _install_ntff_shim():
    """Provide antenv.axon_hooks (absent on this image) so trace=True /
    BASS_TRACE=1 profiling works instead of crashing. Silent no-op on
    any failure."""
    try:
        if "antenv.axon_hooks" in sys.modules:
            return
        m = types.ModuleType("antenv.axon_hooks")
        m._hook = None
        m.set_axon_ntff_profile_hook = lambda h: setattr(m, "_hook", h)
        m.get_axon_ntff_profile_hook = lambda: m._hook
        sys.modules["antenv.axon_hooks"] = m
        import antenv

        antenv.axon_hooks = m
        from trn_agent_boot.trn_boot import _ntff_profile_via_ctypes

        hook = _ntff_profile_via_ctypes("/opt/axon/libaxon_pjrt.so")
        if hook is not None:
            m.set_axon_ntff_profile_hook(hook)
    except Exception:
        pass


_NC_CACHE = {}
LAST_RESULT = None


def _build_nc():
    import concourse.tile as tile
    from concourse import bacc, mybir

    f32 = mybir.dt.float32
    bf16 = mybir.dt.bfloat16
    AF = mybir.ActivationFunctionType
    ALU = mybir.AluOpType

    nc = bacc.Bacc("TRN2", target_bir_lowering=False, debug=False,
                   num_devices=NCORES)

    # ---- DRAM I/O (identical shapes on every core; data differs) ----
    oh_d = nc.dram_tensor("oh", [D, T * 3 * N], bf16, kind="ExternalInput")
    st_d = nc.dram_tensor("st", [NS, T * N], bf16, kind="ExternalInput")
    xm_d = nc.dram_tensor("xm", [D, T], f32, kind="ExternalInput")
    visb_d = nc.dram_tensor("visb", [D, TH * N], f32, kind="ExternalInput")
    a_d = nc.dram_tensor("a4", [4, D, TH * NET * N], bf16,
                         kind="ExternalInput")
    ep_d = nc.dram_tensor("ep", [D, 3 * D], bf16, kind="ExternalInput")
    sp_d = nc.dram_tensor("sp", [NS, D], bf16, kind="ExternalInput")
    c2t_d = nc.dram_tensor("c2t", [D, D], bf16, kind="ExternalInput")
    b1_d = nc.dram_tensor("b1", [D, 1], f32, kind="ExternalInput")
    c2b_d = nc.dram_tensor("c2b", [D, 1], f32, kind="ExternalInput")
    owih_d = nc.dram_tensor("owih", [D, 3 * D], bf16, kind="ExternalInput")
    owhh_d = nc.dram_tensor("owhh", [D, 3 * D], bf16, kind="ExternalInput")
    gwih_d = nc.dram_tensor("gwih", [D, 3 * D], bf16, kind="ExternalInput")
    gwhh_d = nc.dram_tensor("gwhh", [D, 3 * D], bf16, kind="ExternalInput")
    gb_d = nc.dram_tensor("gb", [D, 4], f32, kind="ExternalInput")
    wct_d = nc.dram_tensor("wct", [D, NET * D], bf16, kind="ExternalInput")
    ident_d = nc.dram_tensor("ident", [D, D], f32, kind="ExternalInput")
    visn_d = nc.dram_tensor("visn", [D, TH * 4], f32, kind="ExternalInput")
    outr_d = nc.dram_tensor("outr", [TH, N, D], f32, kind="ExternalOutput")
    outg_d = nc.dram_tensor("outg", [D, TH], f32, kind="ExternalOutput")

    with tile.TileContext(nc) as tc:
        with (
            tc.tile_pool(name="const", bufs=1) as cpool,
            tc.tile_pool(name="wk", bufs=2) as wk,
            tc.tile_pool(name="zsb", bufs=1) as zpool,
            tc.tile_pool(name="pg", bufs=1, space="PSUM") as pg,
            tc.tile_pool(name="pzz", bufs=1, space="PSUM") as pzz,
            tc.tile_pool(name="pa", bufs=2, space="PSUM") as pa,
        ):
            # ---- persistent tiles + param loads ----
            oh_sb = cpool.tile([D, T * 3 * N], bf16)
            st_sb = cpool.tile([NS, T * N], bf16)
            xm_sb = cpool.tile([D, T], f32)
            visb_sb = cpool.tile([D, TH * N], f32)
            a_sb = [cpool.tile([D, TH * NET * N], bf16, tag=f"a{c}",
                               name=f"a_sb{c}")
                    for c in range(4)]
            ep_sb = cpool.tile([D, 3 * D], bf16)
            sp_sb = cpool.tile([NS, D], bf16)
            c2t_sb = cpool.tile([D, D], bf16)
            b1_sb = cpool.tile([D, 1], f32)
            c2b_sb = cpool.tile([D, 1], f32)
            owih_sb = cpool.tile([D, 3 * D], bf16)
            owhh_sb = cpool.tile([D, 3 * D], bf16)
            gwih_sb = cpool.tile([D, 3 * D], bf16)
            gwhh_sb = cpool.tile([D, 3 * D], bf16)
            gb_sb = cpool.tile([D, 4], f32)
            wct_sb = cpool.tile([D, NET * D], bf16)
            ident_sb = cpool.tile([D, D], f32)
            visn_sb = cpool.tile([D, TH * 4], f32)
            xf_sb = cpool.tile([D, T * N], bf16)
            hf = cpool.tile([D, N], f32)
            hb0 = cpool.tile([D, N], bf16)
            hb1 = cpool.tile([D, N], bf16)
            gacc = cpool.tile([D, TH], f32)

            for sb, dr in [
                (oh_sb, oh_d), (st_sb, st_d), (xm_sb, xm_d),
                (visb_sb, visb_d), (ep_sb, ep_d), (sp_sb, sp_d),
                (c2t_sb, c2t_d), (b1_sb, b1_d), (c2b_sb, c2b_d),
                (owih_sb, owih_d), (owhh_sb, owhh_d), (gwih_sb, gwih_d),
                (gwhh_sb, gwhh_d), (gb_sb, gb_d), (wct_sb, wct_d),
                (ident_sb, ident_d), (visn_sb, visn_d),
            ]:
                nc.sync.dma_start(sb[:], dr.ap())
            # A: per (chunk, tt) pieces so tt=0 message passing can start
            # before the whole 9.6MB has landed.
            SEG = NET * N
            for c in range(4):
                for tt in range(TH):
                    nc.sync.dma_start(
                        a_sb[c][:, tt * SEG:(tt + 1) * SEG],
                        a_d.ap()[c, :, tt * SEG:(tt + 1) * SEG],
                    )

            nc.vector.memset(hf[:], 0.0)
            nc.vector.memset(hb0[:], 0.0)
            nc.vector.memset(hb1[:], 0.0)

            # ---- input projection x_t for all 6 program steps ----
            for t in range(T):
                xp = pg.tile([D, N], f32, tag="pr")
                for c in range(3):
                    nc.tensor.matmul(
                        xp[:], ep_sb[:, c * D:(c + 1) * D],
                        oh_sb[:, (t * 3 + c) * N:(t * 3 + c + 1) * N],
                        start=(c == 0), stop=False,
                    )
                nc.tensor.matmul(xp[:], sp_sb[:],
                                 st_sb[:, t * N:(t + 1) * N],
                                 start=False, stop=True)
                x1 = wk.tile([D, N], bf16, tag="x1")
                nc.scalar.activation(x1[:], xp[:], AF.Relu, bias=b1_sb[:])
                xq = pg.tile([D, N], f32, tag="pz")
                nc.tensor.matmul(xq[:], c2t_sb[:], x1[:],
                                 start=True, stop=True)
                x2 = wk.tile([D, N], f32, tag="x2")
                nc.scalar.activation(x2[:], xq[:], AF.Identity,
                                     bias=c2b_sb[:])
                # x-mask (exact zero for th=0 cores' padding steps)
                nc.vector.tensor_scalar_mul(
                    xf_sb[:, t * N:(t + 1) * N], x2[:], xm_sb[:, t:t + 1])

            # ---- recurrence + message passing ----
            for t in range(T):
                # outer GRU: h = gru(x_t, h)  (bias-free)
                pr = pg.tile([D, N], f32, tag="pr")
                pz = pg.tile([D, N], f32, tag="pz")
                pni = pg.tile([D, N], f32, tag="pni")
                pnh = pg.tile([D, N], f32, tag="pnh")
                xt = xf_sb[:, t * N:(t + 1) * N]
                hprev = hb1 if t % 2 == 0 else hb0
                hcur = hb0 if t % 2 == 0 else hb1
                nc.tensor.matmul(pr[:], owih_sb[:, 0:D], xt,
                                 start=True, stop=False)
                nc.tensor.matmul(pr[:], owhh_sb[:, 0:D], hprev[:],
                                 start=False, stop=True)
                nc.tensor.matmul(pz[:], owih_sb[:, D:2 * D], xt,
                                 start=True, stop=False)
                nc.tensor.matmul(pz[:], owhh_sb[:, D:2 * D], hprev[:],
                                 start=False, stop=True)
                nc.tensor.matmul(pni[:], owih_sb[:, 2 * D:3 * D], xt,
                                 start=True, stop=True)
                nc.tensor.matmul(pnh[:], owhh_sb[:, 2 * D:3 * D], hprev[:],
                                 start=True, stop=True)
                r_ = wk.tile([D, N], f32, tag="r")
                z_ = wk.tile([D, N], f32, tag="z")
                nc.scalar.activation(r_[:], pr[:], AF.Sigmoid)
                nc.scalar.activation(z_[:], pz[:], AF.Sigmoid)
                t1 = wk.tile([D, N], f32, tag="t1")
                nc.vector.tensor_mul(t1[:], r_[:], pnh[:])
                s1 = wk.tile([D, N], f32, tag="s1")
                nc.vector.tensor_add(s1[:], t1[:], pni[:])
                nn_ = wk.tile([D, N], f32, tag="nn")
                nc.scalar.activation(nn_[:], s1[:], AF.Tanh)
                d1 = wk.tile([D, N], f32, tag="d1")
                nc.vector.tensor_sub(d1[:], hf[:], nn_[:])
                zd = wk.tile([D, N], f32, tag="zd")
                nc.vector.tensor_mul(zd[:], z_[:], d1[:])
                nc.vector.tensor_add(hf[:], nn_[:], zd[:])
                nc.scalar.activation(hcur[:], hf[:], AF.Copy)

                if t < 3:
                    continue
                tt = t - 3

                # message passing on h
                zt = zpool.tile([N // 4, 4 * NET * D], bf16, tag="zs")
                for c in range(4):
                    for hh in range(2):
                        zp = pzz.tile([N // 4, NET * D // 2], f32)
                        nc.tensor.matmul(
                            zp[:], hcur[:, c * D:(c + 1) * D],
                            wct_sb[:, hh * 500:(hh + 1) * 500],
                            start=True, stop=True)
                        dc = c * NET * D + hh * 500
                        if (c + hh) % 2 == 0:
                            nc.vector.tensor_copy(zt[:, dc:dc + 500], zp[:])
                        else:
                            nc.scalar.activation(zt[:, dc:dc + 500], zp[:],
                                                 AF.Copy)
                ap_ = pa.tile([D, N], f32, tag="ap")
                nmm = 4 * NET
                i = 0
                for c in range(4):
                    for k in range(NET):
                        nc.tensor.matmul(
                            ap_[:], zt[:, c * NET * D + k * D:
                                       c * NET * D + (k + 1) * D],
                            a_sb[c][:, (tt * NET + k) * N:
                                    (tt * NET + k + 1) * N],
                            start=(i == 0), stop=(i == nmm - 1),
                        )
                        i += 1
                af = wk.tile([D, N], bf16, tag="af")
                nc.scalar.activation(af[:], ap_[:], AF.Copy)

                # inner GRU: feat = gru(a, h)  (with biases)
                qr = pg.tile([D, N], f32, tag="pr")
                qz = pg.tile([D, N], f32, tag="pz")
                qni = pg.tile([D, N], f32, tag="pni")
                qnh = pg.tile([D, N], f32, tag="pnh")
                nc.tensor.matmul(qr[:], gwih_sb[:, 0:D], af[:],
                                 start=True, stop=False)
                nc.tensor.matmul(qr[:], gwhh_sb[:, 0:D], hcur[:],
                                 start=False, stop=True)
                nc.tensor.matmul(qz[:], gwih_sb[:, D:2 * D], af[:],
                                 start=True, stop=False)
                nc.tensor.matmul(qz[:], gwhh_sb[:, D:2 * D], hcur[:],
                                 start=False, stop=True)
                nc.tensor.matmul(qni[:], gwih_sb[:, 2 * D:3 * D], af[:],
                                 start=True, stop=True)
                nc.tensor.matmul(qnh[:], gwhh_sb[:, 2 * D:3 * D], hcur[:],
                                 start=True, stop=True)
                gr = wk.tile([D, N], f32, tag="r")
                gz = wk.tile([D, N], f32, tag="z")
                nc.scalar.activation(gr[:], qr[:], AF.Sigmoid,
                                     bias=gb_sb[:, 0:1])
                nc.scalar.activation(gz[:], qz[:], AF.Sigmoid,
                                     bias=gb_sb[:, 1:2])
                hn = wk.tile([D, N], f32, tag="hn")
                nc.scalar.activation(hn[:], qnh[:], AF.Identity,
                                     bias=gb_sb[:, 3:4])
                t2 = wk.tile([D, N], f32, tag="t1")
                nc.vector.tensor_mul(t2[:], gr[:], hn[:])
                s2 = wk.tile([D, N], f32, tag="s1")
                nc.vector.tensor_add(s2[:], t2[:], qni[:])
                gn = wk.tile([D, N], f32, tag="nn")
                nc.scalar.activation(gn[:], s2[:], AF.Tanh,
                                     bias=gb_sb[:, 2:3])
                d2 = wk.tile([D, N], f32, tag="d1")
                nc.vector.tensor_sub(d2[:], hf[:], gn[:])
                zd2 = wk.tile([D, N], f32, tag="zd")
                nc.vector.tensor_mul(zd2[:], gz[:], d2[:])
                feat = wk.tile([D, N], f32, tag="feat")
                nc.vector.tensor_add(feat[:], gn[:], zd2[:])

                # node reprs (feat * vis) and global partial
                # (sum_n feat*vis^2) in two DVE ops
                vist = visb_sb[:, tt * N:(tt + 1) * N]
                reprs = wk.tile([D, N], f32, tag="reprs")
                nc.vector.tensor_mul(reprs[:], feat[:], vist)

                # transpose to node-major, store, and accumulate the
                # global readout sum_n reprs[n,:]*vis[n] on the PE
                gp = pa.tile([D, 1], f32, tag="gp", bufs=1)
                for c in range(4):
                    tp = pa.tile([D, D], f32, tag="ap")
                    nc.tensor.transpose(tp[:], reprs[:, c * D:(c + 1) * D],
                                        ident_sb[:])
                    ro = wk.tile([D, D], f32, tag="ro")
                    nc.scalar.activation(ro[:], tp[:], AF.Copy)
                    nc.sync.dma_start(
                        outr_d.ap()[tt, c * D:(c + 1) * D, :], ro[:])
                    nc.tensor.matmul(gp[:], ro[:],
                                     visn_sb[:, tt * 4 + c:tt * 4 + c + 1],
                                     start=(c == 0), stop=(c == 3))
                nc.scalar.activation(gacc[:, tt:tt + 1], gp[:], AF.Copy)

            nc.sync.dma_start(outg_d.ap(), gacc[:])

    nc.compile()
    return nc


def _prep_inputs(inputs):
    f32 = np.float32
    cn = np.asarray(inputs["class_names"])
    states = np.asarray(inputs["states"], f32)
    edges = np.asarray(inputs["edges"])
    etyp = np.asarray(inputs["edge_types"])
    vis = np.asarray(inputs["visibility"], f32)
    mask = np.asarray(inputs["mask_edges"], f32)
    obj_emb = np.asarray(inputs["obj_emb"], f32)
    state_W = np.asarray(inputs["state_W"], f32)
    state_b = np.asarray(inputs["state_b"], f32)
    c1_W = np.asarray(inputs["c1_W"], f32)
    c1_b = np.asarray(inputs["c1_b"], f32)
    c2_W = np.asarray(inputs["c2_W"], f32)
    c2_b = np.asarray(inputs["c2_b"], f32)
    edge_embed = np.asarray(inputs["edge_embed"], f32)
    g_wih = np.asarray(inputs["g_wih"], f32)
    g_whh = np.asarray(inputs["g_whh"], f32)
    g_bih = np.asarray(inputs["g_bih"], f32)
    g_bhh = np.asarray(inputs["g_bhh"], f32)
    o_wih = np.asarray(inputs["o_wih"], f32)
    o_whh = np.asarray(inputs["o_whh"], f32)

    # folded params (parameter-only algebra)
    c1a, c1b = c1_W[:, :D], c1_W[:, D:]
    ep = (obj_emb @ c1a.T).reshape(3, D, D).transpose(1, 0, 2).reshape(D, 3 * D)
    sp = (c1b @ state_W).T  # [NS, D]
    b1 = (c1_b + c1b @ state_b)[:, None]
    wct = edge_embed.reshape(NET, D, D).transpose(2, 0, 1).reshape(D, NET * D)
    gb = np.stack([
        g_bih[:D] + g_bhh[:D],
        g_bih[D:2 * D] + g_bhh[D:2 * D],
        g_bih[2 * D:],
        g_bhh[2 * D:],
    ], axis=1)

    params = dict(
        ep=ep.astype(BF16), sp=sp.astype(BF16),
        c2t=np.ascontiguousarray(c2_W.T).astype(BF16),
        b1=b1.astype(f32), c2b=c2_b[:, None].astype(f32),
        owih=np.ascontiguousarray(o_wih.T).astype(BF16),
        owhh=np.ascontiguousarray(o_whh.T).astype(BF16),
        gwih=np.ascontiguousarray(g_wih.T).astype(BF16),
        gwhh=np.ascontiguousarray(g_whh.T).astype(BF16),
        gb=gb.astype(f32), wct=wct.astype(BF16),
        ident=np.eye(D, dtype=f32),
    )

    in_maps = []
    t_rep = np.repeat(np.arange(T), E)
    for core in range(NCORES):
        b, th = core // 2, core % 2
        tsel = np.arange(th * TH, th * TH + TH)  # real t's owned

        # one-hot classes, feature(vocab)-major; th=0 pads steps 0..2
        oh = np.zeros((D, T, 3, N), BF16)
        stf = np.zeros((NS, T, N), BF16)
        xm = np.zeros((D, T), f32)
        for ps in range(T):  # program step
            if th == 1 and ps < TH:
                rt = ps  # recurrence prefix: real t=0..2
            elif th == 1:
                rt = ps
            elif th == 0 and ps >= TH:
                rt = ps - TH  # real t = 0..2 at program steps 3..5
            else:
                continue  # th=0 padding steps: stay zero
            onehot = (cn[b, rt][:, None] ==
                      np.arange(NOBJ)[None, :])  # [N, NOBJ]
            oh[:, ps] = onehot.T.reshape(3, D, N).transpose(1, 0, 2)
            stf[:, ps] = states[b, rt].T
            xm[:, ps] = 1.0
        # adjacency counts for owned t's
        a4 = np.zeros((4, D, TH, NET, N), f32)
        cnt = np.zeros((TH, NET, N, N), f32)
        trl = np.repeat(np.arange(TH), E)
        ksel = etyp[b, tsel].reshape(-1)
        srcsel = edges[b, tsel, :, 0].reshape(-1)
        dstsel = edges[b, tsel, :, 1].reshape(-1)
        msel = mask[b, tsel].reshape(-1)
        np.add.at(cnt, (trl, ksel, srcsel, dstsel), msel)
        # [tt,k,src,dst] -> [src, tt, k, dst] -> chunked on src
        a4 = cnt.transpose(2, 0, 1, 3).reshape(4, D, TH * NET * N)

        visb = np.broadcast_to(
            vis[b, tsel][None, :, :], (D, TH, N)).reshape(D, TH * N)

        im = dict(params)
        im["oh"] = oh.reshape(D, T * 3 * N)
        im["st"] = stf.reshape(NS, T * N)
        im["xm"] = xm
        im["visb"] = np.ascontiguousarray(visb, f32)
        visn = vis[b, tsel].reshape(TH * 4, D).T
        im["visn"] = np.ascontiguousarray(visn, f32)
        im["a4"] = np.ascontiguousarray(a4).astype(BF16)
        in_maps.append(im)
    return in_maps, vis


def kernel(**inputs):
    global LAST_RESULT
    _install_ntff_shim()
    from concourse.bass_utils import run_bass_kernel_spmd

    if "nc" not in _NC_CACHE:
        _NC_CACHE["nc"] = _build_nc()
    nc = _NC_CACHE["nc"]

    in_maps, vis = _prep_inputs(inputs)
    res = run_bass_kernel_spmd(nc, in_maps, list(range(NCORES)))
    LAST_RESULT = res

    node_reprs = np.empty((BS, T, N, D), np.float32)
    global_repr = np.empty((BS, T, D), np.float32)
    denom = vis.sum(-1) + 1e-9  # [BS, T]
    for core in range(NCORES):
        b, th = core // 2, core % 2
        out = res.results[core]
        node_reprs[b, th * TH:(th + 1) * TH] = out["outr"]
        gp = out["outg"]  # [D, TH]
        for tt in range(TH):
            t = th * TH + tt
            global_repr[b, t] = gp[:, tt] / denom[b, t]
    return node_reprs, global_repr
